# revision 47
# baseline (speedup 1.0000x reference)
"""Trainium2 Bass kernel for a linear-attention transformer block (fp8).

Model (see reference):
  ln1 -> q/k/v proj -> feature map elu(x)+1 -> linear attention via
  per-head kv summary [d,e] and k-sum [d] -> out proj -> residual ->
  ln2 -> MLP (gelu-tanh) -> residual.

Sharding (8 cores): token-parallel. Core c owns batch c//2, sequence half
c%2 (2048 tokens). Everything is token-local except the attention kv
summary (sum over the full sequence of a batch), which is reduced with a
pairwise AllReduce of a [128, 520] bf16 buffer (16 heads x [64, 65]
(kv | ksum), packed two heads per 128 partitions).

Device layout notes:
 - Activations are token-major [128 tokens, features]; LN statistics and
   the per-token attention normalizer are per-partition scalars there.
 - Matmuls contract over the partition axis, so activation tiles are
   transposed on the PE (128x128 chunks) where a matmul needs them
   feature-major.
 - All six big projections (q/k/v/o, fc, proj) run in fp8e4 with
   perf_mode=DoubleRow (K=256 per matmul, ~1.4x the bf16 PE rate).
   Activations carry a fixed power-of-two scale SA folded into the LN
   rstd / attention normalizer; weights carry per-tensor power-of-two
   scales chosen host-side from their absmax. The product scale is
   divided out on the ScalarE pass that already reads the PSUM result
   (feature-map / gelu / copy), so dequantization costs no extra ops
   except for the token-major o/proj outputs, which add one ScalarE
   copy each before the residual add.
 - The small attention einsums (kv summary, apply) and the PE
   transposes stay bf16; accumulators stay fp32.
 - LN1/LN2 scale/bias are folded into the adjacent weight/bias on the
   host (exact algebra). All effective biases must be zero (true for
   this model's initialization); this kernel asserts that and skips
   bias adds entirely.
"""

import os
import sys
from contextlib import ExitStack

import numpy as np

for _p in ("/opt/trn_rl_repo",):
    if _p not in sys.path:
        sys.path.insert(0, _p)

import ml_dtypes  # noqa: E402

import concourse.bass as bass  # noqa: E402
import concourse.tile as tile  # noqa: E402
from concourse import bacc  # noqa: E402
from concourse import mybir  # noqa: E402
from concourse.masks import make_identity  # noqa: E402

BF16 = mybir.dt.bfloat16
FP32 = mybir.dt.float32
FP8 = mybir.dt.float8e4
AF = mybir.ActivationFunctionType
ALU = mybir.AluOpType
DR = mybir.MatmulPerfMode.DoubleRow

# Model dims (fixed by the problem).
B, S, H = 4, 4096, 1024
NH, HD = 16, 64
MLP = 4096

HC = H // 128    # 8 contraction chunks over hidden dim
HP = HC // 2     # 4 DoubleRow K=256 chunk pairs
FO = H // 128    # 8 feature chunks (q feature-major)
MO = MLP // 128  # 32 mlp chunks
BLK = 512        # tokens per block
TS = BLK // 128  # 128-token subtiles per block

LN_EPS = 1e-5
SA = 4.0         # fp8 scale on activations (lnx / ln2 / attn)


def build_kernel(nc, t_core, n_cores, sc):
    """Emit the per-core program. sc maps weight name -> fp8 scale."""
    nblk = t_core // BLK
    groups = [[2 * i, 2 * i + 1] for i in range(n_cores // 2)]

    x_d = nc.dram_tensor("x", [t_core, H], FP32, kind="ExternalInput")
    qw_d = nc.dram_tensor("qw", [128, HC, FO * 128], FP8, kind="ExternalInput")
    kw_d = nc.dram_tensor("kw", [128, HC, H], FP8, kind="ExternalInput")
    vw_d = nc.dram_tensor("vw", [128, HC, H], FP8, kind="ExternalInput")
    ow_d = nc.dram_tensor("ow", [128, HC, H], FP8, kind="ExternalInput")
    fcw_d = nc.dram_tensor("fcw", [128, MO, HC * 128], FP8, kind="ExternalInput")
    pjw_d = nc.dram_tensor("projw", [128, MO, H], FP8, kind="ExternalInput")
    out_d = nc.dram_tensor("out", [t_core, H], FP32, kind="ExternalOutput")

    # dequant factors for each PSUM product. The residual stream runs at
    # scale c = SA*s_ow (x is pre-scaled by c on the host, the output is
    # descaled by 1/c on the host): the o-proj PSUM lands exactly in
    # residual scale, and s_pjw is pinned to c so the MLP PSUM does too —
    # both residual adds read PSUM raw, with no dequant pass.
    inv_q = 1.0 / (SA * sc["qw"])
    inv_k = 1.0 / (SA * sc["kw"])
    inv_v = 1.0 / (SA * sc["vw"])
    inv_fc = 1.0 / (SA * sc["fcw"])
    inv_o = 1.0 / (SA * sc["ow"])
    inv_pj = 1.0 / sc["projw"]

    with tile.TileContext(nc) as tc, ExitStack() as ctx:
        consts = ctx.enter_context(tc.tile_pool(name="consts", bufs=1))
        wpool = ctx.enter_context(tc.tile_pool(name="wpool", bufs=1))
        acts = ctx.enter_context(tc.tile_pool(name="acts", bufs=2))
        dram = ctx.enter_context(tc.tile_pool(name="dram", bufs=1, space="DRAM"))
        # PSUM is bank-granular: big 4 banks + fcp 2 (fc psums, shared
        # with the transpose staging — disjoint phases) + small 2 = 8.
        psum = ctx.enter_context(tc.tile_pool(name="psum", bufs=2, space="PSUM"))

        # ---- constants ----------------------------------------------------
        ident = consts.tile([128, 128], BF16)
        make_identity(nc, ident)
        eps_ln = consts.tile([128, 1], FP32)
        nc.vector.memset(eps_ln, LN_EPS / (SA * SA))

        # resident weights: q/k/v/o fp8 (1 MB each)
        qw = wpool.tile([128, HC, FO * 128], FP8)
        nc.sync.dma_start(out=qw, in_=qw_d[:, :, :])
        kw = wpool.tile([128, HC, H], FP8)
        nc.sync.dma_start(out=kw, in_=kw_d[:, :, :])
        vw = wpool.tile([128, HC, H], FP8)
        nc.sync.dma_start(out=vw, in_=vw_d[:, :, :])
        ow = wpool.tile([128, HC, H], FP8)
        nc.sync.dma_start(out=ow, in_=ow_d[:, :, :])

        # kv-summary accumulator: [64 d x (64 kv | 1 ksum)] per head,
        # heads (2h, 2h+1) stacked on partitions.
        kvacc = consts.tile([128, 8 * 65], FP32)
        nc.vector.memset(kvacc, 0.0)

        # ln1(x).T stays resident in SBUF (fp8 halves it vs the old bf16
        # spill-to-DRAM scheme): 4 blocks x 4KB/partition.
        lnxT_blocks = [wpool.tile([128, HC, BLK], FP8, name=f"lnxT{i}")
                       for i in range(nblk)]

        def layernorm_scaled(xt, dst):
            """dst = (xt - mean) * (SA / sqrt(var + eps)), cast to bf16."""
            stats = acts.tile([128, 2, 6], FP32, tag="ln_stats", bufs=2)
            nc.vector.bn_stats(out=stats[:, 0, :], in_=xt[:, 0:512])
            nc.vector.bn_stats(out=stats[:, 1, :], in_=xt[:, 512:1024])
            mv = acts.tile([128, 2], FP32, tag="ln_mv", bufs=2)
            nc.vector.bn_aggr(out=mv, in_=stats)
            rstd = acts.tile([128, 1], FP32, tag="ln_rstd", bufs=2)
            # sqrt((var + eps) / SA^2) = sqrt(var + eps) / SA
            nc.scalar.activation(out=rstd, in_=mv[:, 1:2], func=AF.Sqrt,
                                 bias=eps_ln, scale=1.0 / (SA * SA))
            nc.vector.reciprocal(out=rstd, in_=rstd)
            nc.vector.tensor_scalar(out=dst, in0=xt, scalar1=mv[:, 0:1],
                                    scalar2=rstd, op0=ALU.subtract,
                                    op1=ALU.mult)

        # ---- LN1 stats prepass: all 16 tiles' mean/rstd computed up
        # front (PE is idle during the weight loads anyway), so the main
        # pass A applies LN with a single DVE op per tile.
        ntile = nblk * TS
        mean1 = consts.tile([128, ntile], FP32)
        rstd1 = consts.tile([128, ntile], FP32)
        for t in range(ntile):
            xt = acts.tile([128, H], FP32, tag="xin", bufs=3, name="xpre")
            nc.gpsimd.dma_start(out=xt, in_=x_d[t * 128:t * 128 + 128, :])
            stats = acts.tile([128, 2, 6], FP32, tag="ln_stats", bufs=2)
            nc.vector.bn_stats(out=stats[:, 0, :], in_=xt[:, 0:512])
            nc.vector.bn_stats(out=stats[:, 1, :], in_=xt[:, 512:1024])
            mv = acts.tile([128, 2], FP32, tag="ln_mv", bufs=2)
            nc.vector.bn_aggr(out=mv, in_=stats)
            nc.vector.tensor_copy(mean1[:, t:t + 1], mv[:, 0:1])
            nc.scalar.activation(out=rstd1[:, t:t + 1], in_=mv[:, 1:2],
                                 func=AF.Sqrt, bias=eps_ln,
                                 scale=1.0 / (SA * SA))
        nc.vector.reciprocal(out=rstd1, in_=rstd1)

        def transpose_chunks(src_bf16, dstT, ts_idx):
            """PE-transpose [128,1024] token-major -> chunks of dstT
            ([128, HC, BLK], any dtype — the PSUM->SBUF copy casts).
            4 chunks share one PSUM tile so each copy moves [128,512]."""
            for g in range(2):
                pt = psum.tile([128, 512], BF16, tag="fcp", bufs=2,
                               name="pt")
                for i in range(4):
                    hc = g * 4 + i
                    nc.tensor.transpose(pt[:, i * 128:i * 128 + 128],
                                        src_bf16[:, hc * 128:(hc + 1) * 128],
                                        ident)
                dst = dstT[:, g * 4:g * 4 + 4,
                           ts_idx * 128:ts_idx * 128 + 128]
                src = pt.rearrange("p (c m) -> p c m", m=128)
                if g == 0:  # PSUM reads: only DVE/ScalarE have a port
                    nc.vector.tensor_copy(dst, src)
                else:
                    nc.scalar.copy(out=dst, in_=src)

        def feature_map(ps, dst, n, inv):
            """dst = elu(inv*ps)+1 = min(exp(.),1) + relu(.), bf16 out."""
            e = acts.tile([128, n], BF16, tag="fm_e", bufs=3, name="fm_e")
            nc.scalar.activation(out=e, in_=ps, func=AF.Exp, scale=inv)
            r = acts.tile([128, n], BF16, tag="fm_r", bufs=3, name="fm_r")
            nc.scalar.activation(out=r, in_=ps, func=AF.Relu, scale=inv)
            nc.vector.tensor_scalar_min(out=e, in0=e, scalar1=1.0)
            nc.vector.tensor_add(out=dst, in0=e, in1=r)

        # ================== PASS A: ln1, k/v, kv summary ==================
        for blk in range(nblk):
            lnxT = lnxT_blocks[blk]
            for ts in range(TS):
                t = blk * TS + ts
                xt = acts.tile([128, H], FP32, tag="xin", bufs=3)
                r0 = blk * BLK + ts * 128
                nc.gpsimd.dma_start(out=xt, in_=x_d[r0:r0 + 128, :])
                lnx = acts.tile([128, H], BF16, tag="lnx", bufs=3)
                nc.vector.tensor_scalar(out=lnx, in0=xt,
                                        scalar1=mean1[:, t:t + 1],
                                        scalar2=rstd1[:, t:t + 1],
                                        op0=ALU.subtract, op1=ALU.mult)
                transpose_chunks(lnx, lnxT, ts)

            # k, v projections (token-major), k feature map, kv summary
            for ts in range(TS):
                kf = acts.tile([128, H], BF16, tag="kf", bufs=3)
                # v is stored per-head with a ones column appended so the
                # kv summary and the k-sum come out of ONE matmul per head.
                vt = acts.tile([128, NH, 65], BF16, tag="vt", bufs=3)
                nc.vector.memset(vt[:, :, 64:65], 1.0)
                for which in range(2):  # 0 = k, 1 = v
                    wsb = kw if which == 0 else vw
                    for half in range(2):
                        pp = psum.tile([128, 512], FP32, tag="big", bufs=4,
                                       name="pp_kv")
                        for kp in range(HP):
                            nc.tensor.matmul(
                                pp,
                                lhsT=lnxT[:, 2 * kp:2 * kp + 2,
                                          ts * 128:ts * 128 + 128],
                                rhs=wsb[:, 2 * kp:2 * kp + 2,
                                        half * 512:half * 512 + 512],
                                start=(kp == 0), stop=(kp == HP - 1),
                                perf_mode=DR)
                        if which == 0:
                            feature_map(pp, kf[:, half * 512:half * 512 + 512],
                                        512, inv_k)
                        else:
                            dst = vt[:, half * 8:half * 8 + 8, 0:64]
                            src = pp.rearrange("p (c m) -> p c m", m=64)
                            nc.vector.tensor_scalar_mul(dst, in0=src,
                                                        scalar1=inv_v)
                # kv summary: one [64,65] matmul per head, 8 heads per
                # [128,260] psum tile, one accumulate per psum tile.
                for g in range(2):
                    pkv = psum.tile([128, 260], FP32, tag="small", bufs=2,
                                    name="pkv", padded_shape=[128, 512])
                    for i in range(8):
                        h = g * 8 + i
                        r, c = h % 2, (h // 2) - g * 4
                        nc.tensor.matmul(
                            pkv[r * 64:r * 64 + 64, c * 65:c * 65 + 65],
                            lhsT=kf[:, h * HD:h * HD + HD],
                            rhs=vt[:, h, :],
                            start=True, stop=True)
                    nc.vector.tensor_add(
                        out=kvacc[:, g * 260:g * 260 + 260],
                        in0=kvacc[:, g * 260:g * 260 + 260], in1=pkv)

        # ================== AllReduce of kv summary over the seq pair =====
        # The kv (non-ksum) columns are scaled by SA here so the attention
        # apply numerator comes out pre-scaled for the fp8 attnT cast.
        kvacc_bf = consts.tile([128, 8 * 65], BF16)
        nc.vector.tensor_scalar_mul(kvacc_bf, in0=kvacc, scalar1=SA)
        nc.vector.tensor_copy(
            kvacc_bf.rearrange("p (g c) -> p g c", c=65)[:, :, 64:65],
            kvacc.rearrange("p (g c) -> p g c", c=65)[:, :, 64:65])
        cc_in = dram.tile([128, 8 * 65], BF16)
        cc_out = dram.tile([128, 8 * 65], BF16)
        nc.gpsimd.dma_start(out=cc_in, in_=kvacc_bf)
        nc.gpsimd.collective_compute(
            "AllReduce", ALU.add, replica_groups=groups,
            ins=[cc_in.opt()], outs=[cc_out.opt()])
        kvred = consts.tile([128, 8 * 65], BF16)
        nc.gpsimd.dma_start(out=kvred, in_=cc_out)

        # Block-diagonal [d, (e|ksum)] pairs for the apply matmul:
        # rows 0:64 head 2i -> cols 0:65 ; rows 64:128 head 2i+1 -> 65:130
        kvaug = consts.tile([128, 8 * 130], BF16)
        nc.vector.memset(kvaug, 0.0)
        for hp in range(8):
            nc.vector.tensor_copy(
                kvaug[0:64, hp * 130:hp * 130 + 65],
                kvred[0:64, hp * 65:hp * 65 + 65])
            nc.vector.tensor_copy(
                kvaug[64:128, hp * 130 + 65:hp * 130 + 130],
                kvred[64:128, hp * 65:hp * 65 + 65])

        # q projection for one block (emitted so it overlaps the collective
        # when run before the first apply).
        def q_proj(blk):
            lnxT = lnxT_blocks[blk]
            qfT = acts.tile([128, FO * BLK], BF16, tag="qfT", bufs=4,
                            name="qfT")
            for fo in range(FO):
                pp = psum.tile([128, 512], FP32, tag="big", bufs=4,
                               name="pp_q")
                for kp in range(HP):
                    nc.tensor.matmul(
                        pp,
                        lhsT=qw[:, 2 * kp:2 * kp + 2,
                                fo * 128:fo * 128 + 128],
                        rhs=lnxT[:, 2 * kp:2 * kp + 2, 0:BLK],
                        start=(kp == 0), stop=(kp == HP - 1),
                        perf_mode=DR)
                feature_map(pp, qfT[:, fo * BLK:fo * BLK + BLK], BLK, inv_q)
            return qfT

        # all q projections run here: none depend on the collective, so
        # they keep the PE warm while the AllReduce is in flight.
        qfTs = {}
        for blk in range(nblk):
            qfTs[blk] = q_proj(blk)

        # ================== PASS B: apply, o-proj, residual, MLP ==========
        for blk in range(nblk):
            qfT = qfTs.pop(blk)
            attnT = acts.tile([128, HC, BLK], FP8, tag="attnT", bufs=1)
            ln2T = acts.tile([128, HC, BLK], FP8, tag="ln2T", bufs=1)
            xrs = []
            for ts in range(TS):
                # attention apply: 2 head-pairs per [128,260] psum tile
                # ([64 out | denom] x4 head-blocks). The numerator columns
                # of kvaug are pre-scaled by SA (see kvacc_bf), so one
                # broadcast multiply per tile produces attn directly.
                attn = acts.tile([128, H], BF16, tag="attn", bufs=3)
                for g in range(4):
                    pa = psum.tile([128, 260], FP32, tag="small", bufs=2,
                                   name="pa", padded_shape=[128, 512])
                    for i in range(2):
                        hp = g * 2 + i
                        nc.tensor.matmul(
                            pa[:, i * 130:i * 130 + 130],
                            lhsT=qfT[:, hp * BLK + ts * 128:
                                     hp * BLK + ts * 128 + 128],
                            rhs=kvaug[:, hp * 130:hp * 130 + 130],
                            start=True, stop=True)
                    pav = pa.rearrange("p (c m) -> p c m", m=65)
                    rc = acts.tile([128, 4], FP32, tag="rc", bufs=2)
                    nc.vector.reciprocal(
                        out=rc, in_=pav[:, :, 64:65].rearrange(
                            "p c m -> p (c m)"))
                    nc.vector.tensor_tensor(
                        attn.rearrange("p (c m) -> p c m", m=64)
                            [:, g * 4:g * 4 + 4, :],
                        pav[:, :, 0:64],
                        rc[:, :, None].to_broadcast([128, 4, 64]),
                        ALU.mult)
                transpose_chunks(attn, attnT, ts)

            # o-proj + residual + LN2
            for ts in range(TS):
                xt = acts.tile([128, H], FP32, tag="xin", bufs=3,
                               name="xt2")
                r0 = blk * BLK + ts * 128
                nc.gpsimd.dma_start(out=xt, in_=x_d[r0:r0 + 128, :])
                xr = acts.tile([128, H], FP32, tag="xr", bufs=TS + 1,
                               name="xr")
                for half in range(2):
                    pp = psum.tile([128, 512], FP32, tag="big", bufs=4,
                                   name="pp_o")
                    for kp in range(HP):
                        nc.tensor.matmul(
                            pp,
                            lhsT=attnT[:, 2 * kp:2 * kp + 2,
                                       ts * 128:ts * 128 + 128],
                            rhs=ow[:, 2 * kp:2 * kp + 2,
                                   half * 512:half * 512 + 512],
                            start=(kp == 0), stop=(kp == HP - 1),
                            perf_mode=DR)
                    o_bf = acts.tile([128, 512], BF16, tag="o_bf", bufs=2,
                                     name="o_bf")
                    nc.scalar.mul(o_bf, pp, inv_o)
                    nc.vector.tensor_add(
                        out=xr[:, half * 512:half * 512 + 512],
                        in0=xt[:, half * 512:half * 512 + 512],
                        in1=o_bf)
                xrs.append(xr)
                ln2 = acts.tile([128, H], BF16, tag="lnx", bufs=3,
                                name="ln2")
                layernorm_scaled(xr, ln2)
                transpose_chunks(ln2, ln2T, ts)

            # MLP over the whole 512-token block: fc at FD=512 into an
            # fp8 hT staging buffer, then proj in two 256-token groups.
            hTs = acts.tile([128, MO, BLK], FP8, tag="hTs", bufs=1)
            for mo in range(MO):
                fcw_c = acts.tile([128, HC, 128], FP8, tag="fcw_c",
                                  bufs=4)
                nc.sync.dma_start(out=fcw_c, in_=fcw_d[:, mo, :])
                pfc = psum.tile([128, 512], FP32, tag="fcp", bufs=2,
                                name="pfc")
                for kp in range(HP):
                    nc.tensor.matmul(
                        pfc,
                        lhsT=fcw_c[:, 2 * kp:2 * kp + 2, :],
                        rhs=ln2T[:, 2 * kp:2 * kp + 2, 0:BLK],
                        start=(kp == 0), stop=(kp == HP - 1),
                        perf_mode=DR)
                nc.scalar.activation(out=hTs[:, mo, :], in_=pfc,
                                     func=AF.Gelu_apprx_tanh, scale=inv_fc)
            for th in range(2):
                pps = [psum.tile([128, 512], FP32, tag="big", bufs=4,
                                 name=f"pproj_{blk}_{th}_{i}")
                       for i in range(4)]
                for j in range(MO // 2):
                    pjw_c = acts.tile([128, 2, H], FP8, tag="pjw_c", bufs=4)
                    nc.sync.dma_start(out=pjw_c,
                                      in_=pjw_d[:, 2 * j:2 * j + 2, :])
                    for tsl in range(2):
                        for half in range(2):
                            nc.tensor.matmul(
                                pps[tsl * 2 + half],
                                lhsT=hTs[:, 2 * j:2 * j + 2,
                                         th * 256 + tsl * 128:
                                         th * 256 + tsl * 128 + 128],
                                rhs=pjw_c[:, :, half * 512:half * 512 + 512],
                                start=(j == 0), stop=(j == MO // 2 - 1),
                                perf_mode=DR)
                for tsl in range(2):
                    ts_ = th * 2 + tsl
                    pj_bf = acts.tile([128, H], BF16, tag="pj_bf", bufs=2,
                                      name="pj_bf")
                    outt = acts.tile([128, H], FP32, tag="outt", bufs=2)
                    for half in range(2):
                        nc.scalar.mul(pj_bf[:, half * 512:half * 512 + 512],
                                      pps[tsl * 2 + half], inv_pj)
                        nc.vector.tensor_add(
                            out=outt[:, half * 512:half * 512 + 512],
                            in0=xrs[ts_][:, half * 512:half * 512 + 512],
                            in1=pj_bf[:, half * 512:half * 512 + 512])
                    r0 = blk * BLK + ts_ * 128
                    nc.sync.dma_start(out=out_d[r0:r0 + 128, :],
                                      in_=outt)


# ======================= host side =======================================

FP8_NP = ml_dtypes.float8_e4m3  # TRN e4m3: max normal 240


def _pow2_scale(w):
    a = float(np.abs(w).max())
    if a == 0.0 or not np.isfinite(a):
        return 1.0
    return float(2.0 ** np.floor(np.log2(224.0 / a)))


def _to_fp8(w, s):
    return np.clip(w * s, -240.0, 240.0).astype(FP8_NP)


def _prep_weights(inputs):
    """Fold LN affine params into adjacent weights; pre-lay-out for SBUF
    and quantize to fp8e4 with per-tensor power-of-two scales.

    All effective biases must be exactly zero (true for this model's
    initialization: zero biases, ln_b zero) — asserted here.
    """
    f32 = lambda k: np.asarray(inputs[k], np.float32)

    ln1_w, ln1_b = f32("ln1_w"), f32("ln1_b")
    ln2_w, ln2_b = f32("ln2_w"), f32("ln2_b")

    out = {}
    scales = {}

    def put(name, wd):
        s = _pow2_scale(wd)
        out[name] = np.ascontiguousarray(_to_fp8(wd, s))
        scales[name] = s

    def eff_bias(b, ln_b, w):
        return b + ln_b @ w

    q_we = ln1_w[:, None] * f32("q_w")
    k_we = ln1_w[:, None] * f32("k_w")
    v_we = ln1_w[:, None] * f32("v_w")
    for nm, w, b in (("qw", q_we, eff_bias(f32("q_b"), ln1_b, f32("q_w"))),
                     ("kw", k_we, eff_bias(f32("k_b"), ln1_b, f32("k_w"))),
                     ("vw", v_we, eff_bias(f32("v_b"), ln1_b, f32("v_w")))):
        assert not np.any(b != 0.0), f"nonzero effective bias for {nm}"
        if nm == "qw":
            wd = w.reshape(HC, 128, FO, 128).transpose(1, 0, 2, 3) \
                  .reshape(128, HC, FO * 128)
        else:
            wd = w.reshape(HC, 128, H).transpose(1, 0, 2)
        put(nm, wd)

    assert not np.any(f32("o_b") != 0.0)
    put("ow", f32("o_w").reshape(HC, 128, H).transpose(1, 0, 2))

    fce = ln2_w[:, None] * f32("fc_w")
    assert not np.any(eff_bias(f32("fc_b"), ln2_b, f32("fc_w")) != 0.0)
    put("fcw", fce.reshape(HC, 128, MO, 128).transpose(1, 2, 0, 3)
               .reshape(128, MO, HC * 128))

    # proj scale is pinned to the residual-stream scale c = SA*s_ow so the
    # device adds the proj PSUM into the residual without a dequant pass.
    assert not np.any(f32("proj_b") != 0.0)
    pj = f32("proj_w").reshape(MO, 128, H).transpose(1, 0, 2)
    c = SA * scales["ow"]
    assert float(np.abs(pj).max()) * c <= 240.0, "proj overflows fp8 at c"
    out["projw"] = np.ascontiguousarray(_to_fp8(pj, c))
    scales["projw"] = c

    return out, scales


def _run(inputs, trace=False):
    from concourse.bass_utils import run_bass_kernel_spmd

    n_cores = 8
    t_core = B * S // n_cores  # 2048

    wd, scales = _prep_weights(inputs)
    x = np.ascontiguousarray(np.asarray(inputs["x"], np.float32))

    nc = bacc.Bacc(None, num_devices=n_cores, target_bir_lowering=False)
    build_kernel(nc, t_core, n_cores, scales)
    nc.compile()

    half = S // 2
    in_maps = []
    for c in range(n_cores):
        b, sh = c // 2, c % 2
        m = {"x": np.ascontiguousarray(x[b, sh * half:(sh + 1) * half, :])}
        m.update(wd)
        in_maps.append(m)

    res = run_bass_kernel_spmd(nc, in_maps, core_ids=list(range(n_cores)),
                               trace=trace)

    out = np.empty((B, S, H), np.float32)
    for cc in range(n_cores):
        b, sh = cc // 2, cc % 2
        out[b, sh * half:(sh + 1) * half, :] = res.results[cc]["out"]
    return out, res


def kernel(**inputs):
    return _run(inputs)[0]


if __name__ == "__main__":
    os.environ.setdefault("BASS_NEVER_TRACE", "1")
    import reference

    inputs = {k: np.asarray(v) for k, v in reference.setup_inputs().items()}
    got = kernel(**inputs)
    exp = np.asarray(reference.reference(**inputs))
    err = np.abs(got - exp).max() / np.abs(exp).max()
    print("Relative error:", err)


# revision 52
# speedup vs baseline: 1.1534x; 1.1534x over previous
"""Trainium2 Bass kernel for a linear-attention transformer block (fp8).

Model (see reference):
  ln1 -> q/k/v proj -> feature map elu(x)+1 -> linear attention via
  per-head kv summary [d,e] and k-sum [d] -> out proj -> residual ->
  ln2 -> MLP (gelu-tanh) -> residual.

Sharding (8 cores): token-parallel. Core c owns batch c//2, sequence half
c%2 (2048 tokens). Everything is token-local except the attention kv
summary (sum over the full sequence of a batch), which is reduced with a
pairwise AllReduce of a [128, 520] bf16 buffer (16 heads x [64, 65]
(kv | ksum), packed two heads per 128 partitions).

Device layout notes:
 - Activations are token-major [128 tokens, features]; LN statistics and
   the per-token attention normalizer are per-partition scalars there.
 - Matmuls contract over the partition axis, so activation tiles are
   transposed on the PE (128x128 chunks) where a matmul needs them
   feature-major.
 - All six big projections (q/k/v/o, fc, proj) run in fp8e4 with
   perf_mode=DoubleRow (K=256 per matmul, ~1.4x the bf16 PE rate).
   Activations carry a fixed power-of-two scale SA folded into the LN
   rstd / attention normalizer; weights carry per-tensor power-of-two
   scales chosen host-side from their absmax. The product scale is
   divided out on the ScalarE pass that already reads the PSUM result
   (feature-map / gelu / copy), so dequantization costs no extra ops
   except for the token-major o/proj outputs, which add one ScalarE
   copy each before the residual add.
 - The small attention einsums (kv summary, apply) and the PE
   transposes stay bf16; accumulators stay fp32.
 - LN1/LN2 scale/bias are folded into the adjacent weight/bias on the
   host (exact algebra). All effective biases must be zero (true for
   this model's initialization); this kernel asserts that and skips
   bias adds entirely.
"""

import os
import sys
from contextlib import ExitStack

import numpy as np

for _p in ("/opt/trn_rl_repo",):
    if _p not in sys.path:
        sys.path.insert(0, _p)

import ml_dtypes  # noqa: E402

import concourse.bass as bass  # noqa: E402
import concourse.tile as tile  # noqa: E402
from concourse import bacc  # noqa: E402
from concourse import mybir  # noqa: E402
from concourse.masks import make_identity  # noqa: E402

BF16 = mybir.dt.bfloat16
FP32 = mybir.dt.float32
FP8 = mybir.dt.float8e4
AF = mybir.ActivationFunctionType
ALU = mybir.AluOpType
DR = mybir.MatmulPerfMode.DoubleRow

# Model dims (fixed by the problem).
B, S, H = 4, 4096, 1024
NH, HD = 16, 64
MLP = 4096

HC = H // 128    # 8 contraction chunks over hidden dim
HP = HC // 2     # 4 DoubleRow K=256 chunk pairs
FO = H // 128    # 8 feature chunks (q feature-major)
MO = MLP // 128  # 32 mlp chunks
BLK = 512        # tokens per block
TS = BLK // 128  # 128-token subtiles per block

LN_EPS = 1e-5
SA = 4.0         # fp8 scale on activations (lnx / ln2 / attn)


def build_kernel(nc, t_core, n_cores, sc):
    """Emit the per-core program. sc maps weight name -> fp8 scale."""
    nblk = t_core // BLK
    groups = [[2 * i, 2 * i + 1] for i in range(n_cores // 2)]

    x_d = nc.dram_tensor("x", [t_core, H], FP32, kind="ExternalInput")
    qw_d = nc.dram_tensor("qw", [128, HC, FO * 128], FP8, kind="ExternalInput")
    kw_d = nc.dram_tensor("kw", [128, HC, H], FP8, kind="ExternalInput")
    vw_d = nc.dram_tensor("vw", [128, HC, H], FP8, kind="ExternalInput")
    ow_d = nc.dram_tensor("ow", [128, HC, H], FP8, kind="ExternalInput")
    fcw_d = nc.dram_tensor("fcw", [128, MO, HC * 128], FP8, kind="ExternalInput")
    pjw_d = nc.dram_tensor("projw", [128, MO, H], FP8, kind="ExternalInput")
    out_d = nc.dram_tensor("out", [t_core, H], FP32, kind="ExternalOutput")

    # dequant factors for each PSUM product. The residual stream runs at
    # scale c = SA*s_ow (x is pre-scaled by c on the host, the output is
    # descaled by 1/c on the host): the o-proj PSUM lands exactly in
    # residual scale, and s_pjw is pinned to c so the MLP PSUM does too —
    # both residual adds read PSUM raw, with no dequant pass.
    inv_q = 1.0 / (SA * sc["qw"])
    inv_k = 1.0 / (SA * sc["kw"])
    inv_v = 1.0 / (SA * sc["vw"])
    inv_fc = 1.0 / (SA * sc["fcw"])
    inv_o = 1.0 / (SA * sc["ow"])
    inv_pj = 1.0 / sc["projw"]

    with tile.TileContext(nc) as tc, ExitStack() as ctx:
        consts = ctx.enter_context(tc.tile_pool(name="consts", bufs=1))
        wpool = ctx.enter_context(tc.tile_pool(name="wpool", bufs=1))
        acts = ctx.enter_context(tc.tile_pool(name="acts", bufs=2))
        dram = ctx.enter_context(tc.tile_pool(name="dram", bufs=1, space="DRAM"))
        # PSUM is bank-granular: big 4 banks + fcp 2 (fc psums, shared
        # with the transpose staging — disjoint phases) + small 2 = 8.
        psum = ctx.enter_context(tc.tile_pool(name="psum", bufs=2, space="PSUM"))

        # ---- constants ----------------------------------------------------
        ident = consts.tile([128, 128], BF16)
        make_identity(nc, ident)
        eps_ln = consts.tile([128, 1], FP32)
        nc.vector.memset(eps_ln, LN_EPS / (SA * SA))

        # resident weights: q/k/v/o fp8 (1 MB each)
        qw = wpool.tile([128, HC, FO * 128], FP8)
        nc.sync.dma_start(out=qw, in_=qw_d[:, :, :])
        kw = wpool.tile([128, HC, H], FP8)
        nc.sync.dma_start(out=kw, in_=kw_d[:, :, :])
        vw = wpool.tile([128, HC, H], FP8)
        nc.sync.dma_start(out=vw, in_=vw_d[:, :, :])
        ow = wpool.tile([128, HC, H], FP8)
        nc.sync.dma_start(out=ow, in_=ow_d[:, :, :])

        # kv-summary accumulator: [64 d x (64 kv | 1 ksum)] per head,
        # heads (2h, 2h+1) stacked on partitions.
        kvacc = consts.tile([128, 8 * 65], FP32)
        nc.vector.memset(kvacc, 0.0)

        # ln1(x).T stays resident in SBUF (fp8 halves it vs the old bf16
        # spill-to-DRAM scheme): 4 blocks x 4KB/partition.
        lnxT_blocks = [wpool.tile([128, HC, BLK], FP8, name=f"lnxT{i}")
                       for i in range(nblk)]

        def layernorm_scaled(xt, dst):
            """dst = (xt - mean) * (SA / sqrt(var + eps)), cast to bf16."""
            stats = acts.tile([128, 2, 6], FP32, tag="ln_stats", bufs=2)
            nc.vector.bn_stats(out=stats[:, 0, :], in_=xt[:, 0:512])
            nc.vector.bn_stats(out=stats[:, 1, :], in_=xt[:, 512:1024])
            mv = acts.tile([128, 2], FP32, tag="ln_mv", bufs=2)
            nc.vector.bn_aggr(out=mv, in_=stats)
            rstd = acts.tile([128, 1], FP32, tag="ln_rstd", bufs=2)
            # sqrt((var + eps) / SA^2) = sqrt(var + eps) / SA
            nc.scalar.activation(out=rstd, in_=mv[:, 1:2], func=AF.Sqrt,
                                 bias=eps_ln, scale=1.0 / (SA * SA))
            nc.vector.reciprocal(out=rstd, in_=rstd)
            nc.vector.tensor_scalar(out=dst, in0=xt, scalar1=mv[:, 0:1],
                                    scalar2=rstd, op0=ALU.subtract,
                                    op1=ALU.mult)



        def transpose_chunks(src_bf16, dstT, ts_idx, both_dve=False):
            """PE-transpose [128,1024] token-major -> chunks of dstT
            ([128, HC, BLK], any dtype — the PSUM->SBUF copy casts).
            4 chunks share one PSUM tile so each copy moves [128,512].
            PSUM reads can only go to DVE/ScalarE; in pass B ScalarE is
            the loaded engine, so both copies ride DVE there."""
            for g in range(2):
                pt = psum.tile([128, 512], BF16, tag="fcp", bufs=2,
                               name="pt")
                for i in range(4):
                    hc = g * 4 + i
                    nc.tensor.transpose(pt[:, i * 128:i * 128 + 128],
                                        src_bf16[:, hc * 128:(hc + 1) * 128],
                                        ident)
                dst = dstT[:, g * 4:g * 4 + 4,
                           ts_idx * 128:ts_idx * 128 + 128]
                src = pt.rearrange("p (c m) -> p c m", m=128)
                if g == 0 or both_dve:
                    nc.vector.tensor_copy(dst, src)
                else:
                    nc.scalar.copy(out=dst, in_=src)

        def feature_map(ps, dst, n, inv):
            """dst = elu(inv*ps)+1 = min(exp(.),1) + relu(.), bf16 out."""
            e = acts.tile([128, n], BF16, tag="fm_e", bufs=3, name="fm_e")
            nc.scalar.activation(out=e, in_=ps, func=AF.Exp, scale=inv)
            r = acts.tile([128, n], BF16, tag="fm_r", bufs=3, name="fm_r")
            nc.scalar.activation(out=r, in_=ps, func=AF.Relu, scale=inv)
            nc.vector.tensor_scalar_min(out=e, in0=e, scalar1=1.0)
            nc.vector.tensor_add(out=dst, in0=e, in1=r)

        # ================== PASS A: ln1, k/v, kv summary ==================
        for blk in range(nblk):
            lnxT = lnxT_blocks[blk]
            for ts in range(TS):
                xt = acts.tile([128, H], FP32, tag="xin", bufs=3)
                r0 = blk * BLK + ts * 128
                nc.gpsimd.dma_start(out=xt, in_=x_d[r0:r0 + 128, :])
                lnx = acts.tile([128, H], BF16, tag="lnx", bufs=3)
                layernorm_scaled(xt, lnx)
                transpose_chunks(lnx, lnxT, ts)

            # k, v projections (token-major), k feature map, kv summary
            for ts in range(TS):
                kf = acts.tile([128, H], BF16, tag="kf", bufs=3)
                # v is stored per-head with a ones column appended so the
                # kv summary and the k-sum come out of ONE matmul per head.
                vt = acts.tile([128, NH, 65], BF16, tag="vt", bufs=3)
                nc.vector.memset(vt[:, :, 64:65], 1.0)
                for which in range(2):  # 0 = k, 1 = v
                    wsb = kw if which == 0 else vw
                    for half in range(2):
                        pp = psum.tile([128, 512], FP32, tag="big", bufs=4,
                                       name="pp_kv")
                        for kp in range(HP):
                            nc.tensor.matmul(
                                pp,
                                lhsT=lnxT[:, 2 * kp:2 * kp + 2,
                                          ts * 128:ts * 128 + 128],
                                rhs=wsb[:, 2 * kp:2 * kp + 2,
                                        half * 512:half * 512 + 512],
                                start=(kp == 0), stop=(kp == HP - 1),
                                perf_mode=DR)
                        if which == 0:
                            feature_map(pp, kf[:, half * 512:half * 512 + 512],
                                        512, inv_k)
                        else:
                            dst = vt[:, half * 8:half * 8 + 8, 0:64]
                            src = pp.rearrange("p (c m) -> p c m", m=64)
                            nc.vector.tensor_scalar_mul(dst, in0=src,
                                                        scalar1=inv_v)
                # kv summary: one [64,65] matmul per head, 8 heads per
                # [128,260] psum tile, one accumulate per psum tile.
                for g in range(2):
                    pkv = psum.tile([128, 260], FP32, tag="small", bufs=2,
                                    name="pkv", padded_shape=[128, 512])
                    for i in range(8):
                        h = g * 8 + i
                        r, c = h % 2, (h // 2) - g * 4
                        nc.tensor.matmul(
                            pkv[r * 64:r * 64 + 64, c * 65:c * 65 + 65],
                            lhsT=kf[:, h * HD:h * HD + HD],
                            rhs=vt[:, h, :],
                            start=True, stop=True)
                    nc.vector.tensor_add(
                        out=kvacc[:, g * 260:g * 260 + 260],
                        in0=kvacc[:, g * 260:g * 260 + 260], in1=pkv)

        # ================== AllReduce of kv summary over the seq pair =====
        # The kv (non-ksum) columns are scaled by SA here so the attention
        # apply numerator comes out pre-scaled for the fp8 attnT cast.
        kvacc_bf = consts.tile([128, 8 * 65], BF16)
        nc.vector.tensor_scalar_mul(kvacc_bf, in0=kvacc, scalar1=SA)
        nc.vector.tensor_copy(
            kvacc_bf.rearrange("p (g c) -> p g c", c=65)[:, :, 64:65],
            kvacc.rearrange("p (g c) -> p g c", c=65)[:, :, 64:65])
        cc_in = dram.tile([128, 8 * 65], BF16)
        cc_out = dram.tile([128, 8 * 65], BF16)
        nc.gpsimd.dma_start(out=cc_in, in_=kvacc_bf)
        nc.gpsimd.collective_compute(
            "AllReduce", ALU.add, replica_groups=groups,
            ins=[cc_in.opt()], outs=[cc_out.opt()])
        kvred = consts.tile([128, 8 * 65], BF16)
        nc.gpsimd.dma_start(out=kvred, in_=cc_out)

        # Block-diagonal [d, (e|ksum)] pairs for the apply matmul:
        # rows 0:64 head 2i -> cols 0:65 ; rows 64:128 head 2i+1 -> 65:130
        kvaug = consts.tile([128, 8 * 130], BF16)
        nc.vector.memset(kvaug, 0.0)
        for hp in range(8):
            nc.vector.tensor_copy(
                kvaug[0:64, hp * 130:hp * 130 + 65],
                kvred[0:64, hp * 65:hp * 65 + 65])
            nc.vector.tensor_copy(
                kvaug[64:128, hp * 130 + 65:hp * 130 + 130],
                kvred[64:128, hp * 65:hp * 65 + 65])

        # q projection for one block (emitted so it overlaps the collective
        # when run before the first apply).
        def q_proj(blk):
            lnxT = lnxT_blocks[blk]
            qfT = acts.tile([128, FO * BLK], BF16, tag="qfT", bufs=3,
                            name="qfT")
            for fo in range(FO):
                pp = psum.tile([128, 512], FP32, tag="big", bufs=4,
                               name="pp_q")
                for kp in range(HP):
                    nc.tensor.matmul(
                        pp,
                        lhsT=qw[:, 2 * kp:2 * kp + 2,
                                fo * 128:fo * 128 + 128],
                        rhs=lnxT[:, 2 * kp:2 * kp + 2, 0:BLK],
                        start=(kp == 0), stop=(kp == HP - 1),
                        perf_mode=DR)
                feature_map(pp, qfT[:, fo * BLK:fo * BLK + BLK], BLK, inv_q)
            return qfT

        # all q projections run here: none depend on the collective, so
        # they keep the PE warm while the AllReduce is in flight.
        qfTs = {}
        for blk in range(nblk):
            qfTs[blk] = q_proj(blk)

        # ================== PASS B: apply, o-proj, residual, MLP ==========
        for blk in range(nblk):
            qfT = qfTs.pop(blk)
            attnT = acts.tile([128, HC, BLK], FP8, tag="attnT", bufs=2)
            ln2T = acts.tile([128, HC, BLK], FP8, tag="ln2T", bufs=2)
            xrs = []
            for ts in range(TS):
                # attention apply: 2 head-pairs per [128,260] psum tile
                # ([64 out | denom] x4 head-blocks). The numerator columns
                # of kvaug are pre-scaled by SA (see kvacc_bf), so one
                # broadcast multiply per tile produces attn directly.
                attn = acts.tile([128, H], BF16, tag="attn", bufs=3)
                for g in range(4):
                    pa = psum.tile([128, 260], FP32, tag="small", bufs=2,
                                   name="pa", padded_shape=[128, 512])
                    for i in range(2):
                        hp = g * 2 + i
                        nc.tensor.matmul(
                            pa[:, i * 130:i * 130 + 130],
                            lhsT=qfT[:, hp * BLK + ts * 128:
                                     hp * BLK + ts * 128 + 128],
                            rhs=kvaug[:, hp * 130:hp * 130 + 130],
                            start=True, stop=True)
                    pav = pa.rearrange("p (c m) -> p c m", m=65)
                    rc = acts.tile([128, 4], FP32, tag="rc", bufs=2)
                    nc.vector.reciprocal(
                        out=rc, in_=pav[:, :, 64:65].rearrange(
                            "p c m -> p (c m)"))
                    nc.vector.tensor_tensor(
                        attn.rearrange("p (c m) -> p c m", m=64)
                            [:, g * 4:g * 4 + 4, :],
                        pav[:, :, 0:64],
                        rc[:, :, None].to_broadcast([128, 4, 64]),
                        ALU.mult)
                transpose_chunks(attn, attnT, ts, both_dve=True)

            # o-proj + residual + LN2
            for ts in range(TS):
                xt = acts.tile([128, H], FP32, tag="xin", bufs=3,
                               name="xt2")
                r0 = blk * BLK + ts * 128
                nc.gpsimd.dma_start(out=xt, in_=x_d[r0:r0 + 128, :])
                xr = acts.tile([128, H], FP32, tag="xr", bufs=TS + 1,
                               name="xr")
                for half in range(2):
                    pp = psum.tile([128, 512], FP32, tag="big", bufs=4,
                                   name="pp_o")
                    for kp in range(HP):
                        nc.tensor.matmul(
                            pp,
                            lhsT=attnT[:, 2 * kp:2 * kp + 2,
                                       ts * 128:ts * 128 + 128],
                            rhs=ow[:, 2 * kp:2 * kp + 2,
                                   half * 512:half * 512 + 512],
                            start=(kp == 0), stop=(kp == HP - 1),
                            perf_mode=DR)
                    o_bf = acts.tile([128, 512], BF16, tag="o_bf", bufs=2,
                                     name="o_bf")
                    nc.vector.tensor_scalar_mul(o_bf, in0=pp, scalar1=inv_o)
                    nc.vector.tensor_add(
                        out=xr[:, half * 512:half * 512 + 512],
                        in0=xt[:, half * 512:half * 512 + 512],
                        in1=o_bf)
                xrs.append(xr)
                ln2 = acts.tile([128, H], BF16, tag="lnx", bufs=3,
                                name="ln2")
                layernorm_scaled(xr, ln2)
                transpose_chunks(ln2, ln2T, ts, both_dve=True)

            # MLP over the whole 512-token block: fc at FD=512 into an
            # fp8 hT staging buffer, then proj in two 256-token groups.
            hTs = acts.tile([128, MO, BLK], FP8, tag="hTs", bufs=1)
            for mo in range(MO):
                if mo % 2 == 0:
                    fcw_c = acts.tile([128, 2, HC, 128], FP8, tag="fcw_c",
                                      bufs=4)
                    nc.sync.dma_start(out=fcw_c,
                                      in_=fcw_d[:, mo:mo + 2, :])
                pfc = psum.tile([128, 512], FP32, tag="fcp", bufs=2,
                                name="pfc")
                for kp in range(HP):
                    nc.tensor.matmul(
                        pfc,
                        lhsT=fcw_c[:, mo % 2, 2 * kp:2 * kp + 2, :],
                        rhs=ln2T[:, 2 * kp:2 * kp + 2, 0:BLK],
                        start=(kp == 0), stop=(kp == HP - 1),
                        perf_mode=DR)
                nc.scalar.activation(out=hTs[:, mo, :], in_=pfc,
                                     func=AF.Gelu_apprx_tanh, scale=inv_fc)
            for th in range(2):
                pps = [psum.tile([128, 512], FP32, tag="big", bufs=4,
                                 name=f"pproj_{blk}_{th}_{i}")
                       for i in range(4)]
                for j in range(MO // 2):
                    pjw_c = acts.tile([128, 2, H], FP8, tag="pjw_c", bufs=4)
                    nc.sync.dma_start(out=pjw_c,
                                      in_=pjw_d[:, 2 * j:2 * j + 2, :])
                    for tsl in range(2):
                        for half in range(2):
                            nc.tensor.matmul(
                                pps[tsl * 2 + half],
                                lhsT=hTs[:, 2 * j:2 * j + 2,
                                         th * 256 + tsl * 128:
                                         th * 256 + tsl * 128 + 128],
                                rhs=pjw_c[:, :, half * 512:half * 512 + 512],
                                start=(j == 0), stop=(j == MO // 2 - 1),
                                perf_mode=DR)
                for tsl in range(2):
                    ts_ = th * 2 + tsl
                    pj_bf = acts.tile([128, H], BF16, tag="pj_bf", bufs=2,
                                      name="pj_bf")
                    outt = acts.tile([128, H], FP32, tag="outt", bufs=2)
                    for half in range(2):
                        nc.vector.tensor_scalar_mul(
                            pj_bf[:, half * 512:half * 512 + 512],
                            in0=pps[tsl * 2 + half], scalar1=inv_pj)
                        nc.vector.tensor_add(
                            out=outt[:, half * 512:half * 512 + 512],
                            in0=xrs[ts_][:, half * 512:half * 512 + 512],
                            in1=pj_bf[:, half * 512:half * 512 + 512])
                    r0 = blk * BLK + ts_ * 128
                    nc.sync.dma_start(out=out_d[r0:r0 + 128, :],
                                      in_=outt)


# ======================= host side =======================================

FP8_NP = ml_dtypes.float8_e4m3  # TRN e4m3: max normal 240


def _pow2_scale(w):
    a = float(np.abs(w).max())
    if a == 0.0 or not np.isfinite(a):
        return 1.0
    return float(2.0 ** np.floor(np.log2(224.0 / a)))


def _to_fp8(w, s):
    return np.clip(w * s, -240.0, 240.0).astype(FP8_NP)


def _prep_weights(inputs):
    """Fold LN affine params into adjacent weights; pre-lay-out for SBUF
    and quantize to fp8e4 with per-tensor power-of-two scales.

    All effective biases must be exactly zero (true for this model's
    initialization: zero biases, ln_b zero) — asserted here.
    """
    f32 = lambda k: np.asarray(inputs[k], np.float32)

    ln1_w, ln1_b = f32("ln1_w"), f32("ln1_b")
    ln2_w, ln2_b = f32("ln2_w"), f32("ln2_b")

    out = {}
    scales = {}

    def put(name, wd):
        s = _pow2_scale(wd)
        out[name] = np.ascontiguousarray(_to_fp8(wd, s))
        scales[name] = s

    def eff_bias(b, ln_b, w):
        return b + ln_b @ w

    q_we = ln1_w[:, None] * f32("q_w")
    k_we = ln1_w[:, None] * f32("k_w")
    v_we = ln1_w[:, None] * f32("v_w")
    for nm, w, b in (("qw", q_we, eff_bias(f32("q_b"), ln1_b, f32("q_w"))),
                     ("kw", k_we, eff_bias(f32("k_b"), ln1_b, f32("k_w"))),
                     ("vw", v_we, eff_bias(f32("v_b"), ln1_b, f32("v_w")))):
        assert not np.any(b != 0.0), f"nonzero effective bias for {nm}"
        if nm == "qw":
            wd = w.reshape(HC, 128, FO, 128).transpose(1, 0, 2, 3) \
                  .reshape(128, HC, FO * 128)
        else:
            wd = w.reshape(HC, 128, H).transpose(1, 0, 2)
        put(nm, wd)

    assert not np.any(f32("o_b") != 0.0)
    put("ow", f32("o_w").reshape(HC, 128, H).transpose(1, 0, 2))

    fce = ln2_w[:, None] * f32("fc_w")
    assert not np.any(eff_bias(f32("fc_b"), ln2_b, f32("fc_w")) != 0.0)
    put("fcw", fce.reshape(HC, 128, MO, 128).transpose(1, 2, 0, 3)
               .reshape(128, MO, HC * 128))

    # proj scale is pinned to the residual-stream scale c = SA*s_ow so the
    # device adds the proj PSUM into the residual without a dequant pass.
    assert not np.any(f32("proj_b") != 0.0)
    pj = f32("proj_w").reshape(MO, 128, H).transpose(1, 0, 2)
    c = SA * scales["ow"]
    assert float(np.abs(pj).max()) * c <= 240.0, "proj overflows fp8 at c"
    out["projw"] = np.ascontiguousarray(_to_fp8(pj, c))
    scales["projw"] = c

    return out, scales


def _run(inputs, trace=False):
    from concourse.bass_utils import run_bass_kernel_spmd

    n_cores = 8
    t_core = B * S // n_cores  # 2048

    wd, scales = _prep_weights(inputs)
    x = np.ascontiguousarray(np.asarray(inputs["x"], np.float32))

    nc = bacc.Bacc(None, num_devices=n_cores, target_bir_lowering=False)
    build_kernel(nc, t_core, n_cores, scales)
    nc.compile()

    half = S // 2
    in_maps = []
    for c in range(n_cores):
        b, sh = c // 2, c % 2
        m = {"x": np.ascontiguousarray(x[b, sh * half:(sh + 1) * half, :])}
        m.update(wd)
        in_maps.append(m)

    res = run_bass_kernel_spmd(nc, in_maps, core_ids=list(range(n_cores)),
                               trace=trace)

    out = np.empty((B, S, H), np.float32)
    for cc in range(n_cores):
        b, sh = cc // 2, cc % 2
        out[b, sh * half:(sh + 1) * half, :] = res.results[cc]["out"]
    return out, res


def kernel(**inputs):
    return _run(inputs)[0]


if __name__ == "__main__":
    os.environ.setdefault("BASS_NEVER_TRACE", "1")
    import reference

    inputs = {k: np.asarray(v) for k, v in reference.setup_inputs().items()}
    got = kernel(**inputs)
    exp = np.asarray(reference.reference(**inputs))
    err = np.abs(got - exp).max() / np.abs(exp).max()
    print("Relative error:", err)
